# revision 1
# baseline (speedup 1.0000x reference)
"""DenseGATConv Trainium2 kernel v2 (8 NeuronCores, SPMD, column-sharded).

Same math as v1 (see kernel.py docstring):
    u_i = exp(0.2 a_src_i), e_i = exp(a_src_i), q_j = exp(0.8 a_dst_j)
    M[i,j] = adj[i,j] * max(e_i q_j, u_i)
    out[j,:] = (M^T h)[j,:] / colsum(M)[j] + bias.

v2 engine/schedule changes (driven by timeline-sim engine occupancy):
  - adj DMA'd in 1MB quad-tile chunks interleaved with the xT chunks on the
    SP queue so the adjacency stream is continuous and the tail chunk small.
  - mask-mult (t2 * adj) done as one tensor_tensor per 4-tile span: fewer
    DVE instructions, same 2x DVE mode, lower per-op overhead.
  - h/a_src PSUM->SBUF copies moved from ACT to the idle Pool (gpsimd)
    engine so ACT never serializes the h pipeline (exp groups + q_rep only).
  - a_src fused as a 129th column of the W matmul (no separate PE columns).
  - a_dst path in fp8 (q_j errors cancel between numerator and column sum).
  - numT exported fp16 (den stays f32); halves the output DMA.
"""

import numpy as np
import ml_dtypes
from contextlib import ExitStack

import concourse.bass as bass
import concourse.bacc as bacc
import concourse.tile as tile
from concourse import mybir
from concourse.bass_utils import run_bass_kernel_spmd

F32 = mybir.dt.float32
F16 = mybir.dt.float16
F8 = mybir.dt.float8e4
ALU = mybir.AluOpType
ACTF = mybir.ActivationFunctionType

N, C_IN, C_OUT = 8192, 256, 128
NCORES = 8
JB = N // NCORES          # 1024 destination columns per core
NT = N // 128             # 64 i-tiles
QUAD = 4                  # i-tiles per adj DMA chunk / per mask-mult op
NQ = NT // QUAD           # 16 quads
GRP = 8                   # a_src exp-group size (i-tiles)
XB = 16                   # i-tiles per xT chunk
NXC = NT // XB            # 4 xT chunks

ABLATE = None
DEN_SHIFT_Q = (1, 3, 5, 7, 9, 11, 13, 15)
_nc_cache = {}


def _make_pools(tc, ctx):
    return dict(
        const=ctx.enter_context(tc.tile_pool(name="const", bufs=2)),
        xt_pool=ctx.enter_context(tc.tile_pool(name="xt", bufs=5)),
        h_pool=ctx.enter_context(tc.tile_pool(name="h", bufs=96)),
        scratch=ctx.enter_context(tc.tile_pool(name="scr", bufs=2)),
        adj_pool=ctx.enter_context(tc.tile_pool(name="adj", bufs=6)),
        t2_pool=ctx.enter_context(tc.tile_pool(name="t2", bufs=2)),
        m_pool=ctx.enter_context(tc.tile_pool(name="m", bufs=3)),
        dsum_pool=ctx.enter_context(tc.tile_pool(name="dsum", bufs=1)),
        ps_h=ctx.enter_context(tc.tile_pool(name="psh", bufs=2, space="PSUM")),
        ps_acc=ctx.enter_context(tc.tile_pool(name="psacc", bufs=1,
                                              space="PSUM")),
        ps_pre=ctx.enter_context(tc.tile_pool(name="pspre", bufs=1,
                                              space="PSUM")),
    )


def _emit_body(tc, nc, pools, tensors, rep):
    (xT_in, xTloc_in, adj_in, W_in, att_rep_in,
     numT_out, den_out) = tensors

    adj_r = adj_in.rearrange("(c a p) j -> c p a j", a=QUAD, p=128)

    const = pools["const"]
    xt_pool = pools["xt_pool"]
    h_pool = pools["h_pool"]
    scratch = pools["scratch"]
    adj_pool = pools["adj_pool"]
    t2_pool = pools["t2_pool"]
    m_pool = pools["m_pool"]
    dsum_pool = pools["dsum_pool"]
    ps_h = pools["ps_h"]
    ps_acc = pools["ps_acc"]
    ps_pre = pools["ps_pre"]

    # ---- front-loaded DMAs (SP queue, program order == stream order) ----
    # small constants first, then xTloc (a_dst path), then xc0, adj0, xc1,
    # adj1, ... so q_rep and the first h/exp groups are ready when the first
    # adjacency quads land.
    W_sb = const.tile([128, 258], F16, tag="W_sb", name=f"W_sb_{rep}")      # [k*129 .. ] cols,
    W_view = W_sb[:].rearrange("p (two c) -> p two c", two=2)[:, :, 0:128]
    nc.sync.dma_start(W_view, W_in[:].rearrange("p (two c) -> p two c", two=2))

    xc = [xt_pool.tile([128, 2 * XB * 128], F16, tag="xtc", name=f"xc{cx}_{rep}")
          for cx in range(NXC)]

    xT_v = xT_in[:].rearrange("(two p) n -> p two n", two=2)

    def emit_xc_dma(cx):
        nc.sync.dma_start(
            xc[cx][:].rearrange("p (two n) -> p two n", two=2),
            xT_v[:, :, cx * XB * 128:(cx + 1) * XB * 128])

    emit_xc_dma(0)
    att2 = const.tile([128, 2 * C_OUT], F32, tag="att2", name=f"att2_{rep}")  # attsrc | attdst
    nc.sync.dma_start(att2[:], att_rep_in[:])
    xl8 = const.tile([128, 2 * JB], F16, tag="xl8", name=f"xl8_{rep}")      # k0 | k1 halves
    nc.sync.dma_start(
        xl8[:].rearrange("p (two j) -> p two j", two=2),
        xTloc_in[:].rearrange("(two p) j -> p two j", two=2))
    attsrc = att2[:, 0:C_OUT]
    attdst = att2[:, C_OUT:2 * C_OUT]

    adj_tiles = []

    def emit_adj_dma(q, split=False):
        adj_q = adj_pool.tile([128, QUAD * JB], F16, tag="adj",
                              name=f"adj{q}_{rep}")
        if ABLATE == "dma":
            nc.sync.dma_start(adj_q[:, 0:8], adj_r[q][:, 0:1, 0:8])
            adj_tiles.append(adj_q)
            return
        if split:
            half = adj_r[q][:, 0:QUAD // 2, :]
            nc.sync.dma_start(adj_q[:, 0:QUAD * JB // 2], half)
            nc.sync.dma_start(adj_q[:, QUAD * JB // 2:],
                              adj_r[q][:, QUAD // 2:QUAD, :])
        else:
            nc.sync.dma_start(adj_q[:], adj_r[q])
        adj_tiles.append(adj_q)

    # interleave adj and xc chunks; first quad split for an early start
    emit_adj_dma(0, split=True)
    emit_adj_dma(1)
    emit_xc_dma(1)
    emit_adj_dma(2)
    emit_adj_dma(3)
    emit_xc_dma(2)
    emit_adj_dma(4)
    emit_adj_dma(5)
    emit_xc_dma(3)
    # adj quads 6..15 emitted in the main loop (pool bufs gate prefetch)

    # ---- device-side constants ----
    ones_col = const.tile([128, 1], F16, tag="ones_col", name=f"ones_col_{rep}")
    nc.vector.memset(ones_col[:], 1.0)
    ones_row = const.tile([1, 128], F32, tag="ones_row", name=f"ones_row_{rep}")
    nc.vector.memset(ones_row[:], 1.0)

    # wsrc[k] = sum_c W[k-block, c] att_src[c]; wdst likewise (DVE STT with
    # free-dim accumulate), then cast into W_sb col / fp8.
    wsrc = const.tile([128, 2], F32, tag="wsrc", name=f"wsrc_{rep}")
    wdst = const.tile([128, 2], F32, tag="wdst", name=f"wdst_{rep}")
    for k in range(2):
        sc = scratch.tile([128, C_OUT], F32, tag="scr", name=f"scs{k}_{rep}")
        nc.vector.scalar_tensor_tensor(
            sc[:], W_sb[:, k * 129:k * 129 + 128], 1.0, attsrc,
            op0=ALU.mult, op1=ALU.mult, accum_out=wsrc[:, k:k + 1])
        sc2 = scratch.tile([128, C_OUT], F32, tag="scr", name=f"scd{k}_{rep}")
        nc.vector.scalar_tensor_tensor(
            sc2[:], W_sb[:, k * 129:k * 129 + 128], 1.0, attdst,
            op0=ALU.mult, op1=ALU.mult, accum_out=wdst[:, k:k + 1])
    for k in range(2):
        nc.vector.tensor_copy(W_sb[:, k * 129 + 128:k * 129 + 129],
                              wsrc[:, k:k + 1])
    wdst8 = const.tile([128, 2], F16, tag="wdst8", name=f"wdst8_{rep}")
    nc.vector.tensor_copy(wdst8[:], wdst[:])

    # ---- a_dst path -> q_rep (all fp8; q_j error cancels columnwise) ----
    adst_row = const.tile([1, JB], F32, tag="adst_row", name=f"adst_row_{rep}")
    for hf in range(2):
        ap = ps_pre.tile([1, 512], F32, tag="adst", name=f"adstp{hf}_{rep}")
        for k in range(2):
            nc.tensor.matmul(ap[:],
                             lhsT=wdst8[:, k:k + 1],
                             rhs=xl8[:, k * JB + hf * 512:k * JB + (hf + 1) * 512],
                             start=(k == 0), stop=(k == 1))
        nc.scalar.copy(adst_row[0:1, hf * 512:(hf + 1) * 512], ap[:])
    q_rep = const.tile([128, JB], F16, tag="q_rep", name=f"q_rep_{rep}")
    for hf in range(2):
        qp = ps_pre.tile([128, 512], F32, tag="qrep", name=f"qp{hf}_{rep}")
        nc.tensor.matmul(qp[:], lhsT=ones_row[:],
                         rhs=adst_row[0:1, hf * 512:(hf + 1) * 512],
                         start=True, stop=True)
        nc.scalar.activation(q_rep[:, hf * 512:(hf + 1) * 512], qp[:],
                             ACTF.Exp, scale=0.8)

    # ---- h tiles + a_src (PE matmul w/ fused wsrc col; Pool copies) ----
    h_tiles = []
    asrc_g = [const.tile([128, GRP], F32, tag=f"asrc{g}", name=f"asrc{g}_{rep}")
              for g in range(NT // GRP)]
    ea_g = [const.tile([128, GRP], F32, tag=f"ea{g}", name=f"ea{g}_{rep}")
            for g in range(NT // GRP)]   # exp(a_src)
    u_g = [const.tile([128, GRP], F32, tag=f"u{g}", name=f"u{g}_{rep}")
           for g in range(NT // GRP)]    # exp(0.2 a_src)
    for t in range(NT):
        cx, ti = divmod(t, XB)
        g, gi = divmod(t, GRP)
        hp = ps_h.tile([128, 129], F32, tag="hps", name=f"hps{t}_{rep}")
        for k in range(2):
            nc.tensor.matmul(
                hp[:],
                lhsT=xc[cx][:, k * XB * 128 + ti * 128:
                            k * XB * 128 + (ti + 1) * 128],
                rhs=W_sb[:, k * 129:(k + 1) * 129],
                start=(k == 0), stop=(k == 1))
        h_t = h_pool.tile([128, 129], F16, tag="h", name=f"h{t}_{rep}")
        nc.scalar.copy(h_t[:], hp[:])
        nc.gpsimd.tensor_copy(asrc_g[g][:, gi:gi + 1], h_t[:, 128:129])
        h_tiles.append(h_t)
        if gi == GRP - 1:
            nc.scalar.activation(ea_g[g][:], asrc_g[g][:], ACTF.Exp,
                                 scale=1.0)
            nc.scalar.activation(u_g[g][:], asrc_g[g][:], ACTF.Exp,
                                 scale=0.2)

    # ---- main masked-matmul loop (quad granularity) ----
    num_ps = [ps_acc.tile([C_OUT, 512], F32, tag=f"nps{hf}", name=f"nps{hf}_{rep}")
              for hf in range(2)]
    den_ps = [ps_acc.tile([1, 512], F32, tag=f"dps{hf}", name=f"dps{hf}_{rep}")
              for hf in range(2)]
    DEN_SHIFT = set(DEN_SHIFT_Q)   # quads with den on DVE
    half = QUAD * JB // 2
    for q in range(NQ):
        if q + 6 < NQ:
            emit_adj_dma(q + 6)
        adj_q = adj_tiles[q]
        t2_q = t2_pool.tile([128, QUAD * JB], F16, tag="t2", name=f"t2_{q}_{rep}")
        for a in range(QUAD) if ABLATE != "dve" else []:
            t = q * QUAD + a
            g, gi = divmod(t, GRP)
            nc.vector.tensor_scalar(
                t2_q[:, a * JB:(a + 1) * JB], q_rep[:],
                ea_g[g][:, gi:gi + 1], u_g[g][:, gi:gi + 1],
                op0=ALU.mult, op1=ALU.max)
        m_q = m_pool.tile([128, QUAD * JB], F16, tag="m", name=f"m{q}_{rep}")
        if ABLATE == "dve":
            nc.vector.memset(m_q[:, 0:1], 1.0)
        elif q == 0 or q == NQ - 1:
            nc.vector.tensor_tensor(m_q[:, 0:half], t2_q[:, 0:half],
                                    adj_q[:, 0:half], op=ALU.mult)
            nc.vector.tensor_tensor(m_q[:, half:], t2_q[:, half:],
                                    adj_q[:, half:], op=ALU.mult)
        else:
            nc.vector.tensor_tensor(m_q[:], t2_q[:], adj_q[:], op=ALU.mult)
        _arange = range(QUAD)
        if ABLATE == "pe":
            _arange = [0] if q == 0 else ([QUAD - 1] if q == NQ - 1 else [])
        for a in _arange:
            t = q * QUAD + a
            for hf in range(2):
                ms = m_q[:, a * JB + hf * 512:a * JB + (hf + 1) * 512]
                nc.tensor.matmul(num_ps[hf][:], lhsT=h_tiles[t][:, 0:128],
                                 rhs=ms, start=(t == 0), stop=(t == NT - 1))
                if q not in DEN_SHIFT or ABLATE == "dve":
                    nc.tensor.matmul(den_ps[hf][:], lhsT=ones_col[:], rhs=ms,
                                     start=(t == 0), stop=(t == NT - 1))
        if q in DEN_SHIFT and ABLATE != "dve":
            # den contribution of this quad: sum the 4 m tiles on DVE, then
            # one PE reduction per half on the summed tile
            s01 = dsum_pool.tile([128, JB], F16, tag="s01", name=f"s01_{q}_{rep}")
            nc.vector.tensor_tensor(s01[:], m_q[:, 0:JB], m_q[:, JB:2 * JB],
                                    op=ALU.add)
            s23 = dsum_pool.tile([128, JB], F16, tag="s23", name=f"s23_{q}_{rep}")
            nc.vector.tensor_tensor(s23[:], m_q[:, 2 * JB:3 * JB],
                                    m_q[:, 3 * JB:4 * JB], op=ALU.add)
            s = dsum_pool.tile([128, JB], F16, tag="s", name=f"s_{q}_{rep}")
            nc.vector.tensor_tensor(s[:], s01[:], s23[:], op=ALU.add)
            for hf in range(2):
                nc.tensor.matmul(den_ps[hf][:], lhsT=ones_col[:],
                                 rhs=s[:, hf * 512:(hf + 1) * 512],
                                 start=False, stop=False,
                                 skip_group_check=True)

    # ---- epilogue ----
    num_sb = const.tile([C_OUT, JB], F16, tag="num_sb", name=f"num_sb_{rep}")
    den_sb = const.tile([1, JB], F32, tag="den_sb", name=f"den_sb_{rep}")
    nc.scalar.copy(num_sb[:, 0:512], num_ps[0][:])
    nc.vector.tensor_copy(num_sb[:, 512:1024], num_ps[1][:])
    nc.scalar.copy(den_sb[0:1, 0:512], den_ps[0][:])
    nc.vector.tensor_copy(den_sb[0:1, 512:1024], den_ps[1][:])
    nc.sync.dma_start(numT_out[:], num_sb[:])
    nc.sync.dma_start(den_out[:], den_sb[:])


def build_nc(reps=1):
    key = ("nc", reps)
    if key in _nc_cache:
        return _nc_cache[key]
    nc = bacc.Bacc("TRN2", target_bir_lowering=False, debug=False,
                   num_devices=NCORES)

    xT_in = nc.dram_tensor("xT", [C_IN, N], F16, kind="ExternalInput")
    xTloc_in = nc.dram_tensor("xTloc", [C_IN, JB], F16, kind="ExternalInput")
    adj_in = nc.dram_tensor("adjc", [N, JB], F16, kind="ExternalInput")
    W_in = nc.dram_tensor("Wt", [128, C_IN], F16, kind="ExternalInput")
    att_rep_in = nc.dram_tensor("att_rep", [128, 2 * C_OUT], F32,
                                kind="ExternalInput")

    numT_out = nc.dram_tensor("numT", [C_OUT, JB], F16, kind="ExternalOutput")
    den_out = nc.dram_tensor("den", [1, JB], F32, kind="ExternalOutput")

    tensors = (xT_in, xTloc_in, adj_in, W_in, att_rep_in,
               numT_out, den_out)

    UNROLL = 4
    with tile.TileContext(nc) as tc:
        with ExitStack() as pctx:
            pools = _make_pools(tc, pctx)
            if reps >= 2 * UNROLL:
                n_loop, n_rem = divmod(reps, UNROLL)
                with tc.For_i(0, n_loop, 1, hint_engines=(
                        mybir.EngineType.PE, mybir.EngineType.DVE,
                        mybir.EngineType.Activation, mybir.EngineType.SP,
                        mybir.EngineType.Pool)):
                    for r in range(UNROLL):
                        _emit_body(tc, nc, pools, tensors, r)
                for r in range(n_rem):
                    _emit_body(tc, nc, pools, tensors, UNROLL + r)
            else:
                for r in range(reps):
                    _emit_body(tc, nc, pools, tensors, r)

    nc.compile()
    _nc_cache[key] = nc
    return nc


def make_in_maps(x, adj, W, att_src, att_dst):
    f8 = ml_dtypes.float8_e4m3
    xT = np.ascontiguousarray(x.T.astype(np.float32, copy=False)).astype(
        np.float16)
    Wt = np.ascontiguousarray(
        np.concatenate([W[0:128, :], W[128:256, :]], axis=1)).astype(
        np.float16)
    att_rep = np.ascontiguousarray(np.concatenate([
        np.broadcast_to(att_src.astype(np.float32), (128, C_OUT)),
        np.broadcast_to(att_dst.astype(np.float32), (128, C_OUT))], axis=1))
    in_maps = []
    for d in range(NCORES):
        adj_d = np.ascontiguousarray(
            adj[:, d * JB:(d + 1) * JB].astype(np.float32, copy=False))
        idx = np.arange(JB)
        adj_d[d * JB + idx, idx] = 1.0          # self loops
        adj_d = adj_d.astype(np.float16)        # 0/1: exact
        xTloc = np.ascontiguousarray(xT[:, d * JB:(d + 1) * JB])
        in_maps.append({
            "xT": xT, "xTloc": xTloc, "adjc": adj_d, "Wt": Wt,
            "att_rep": att_rep,
        })
    return in_maps


def postprocess(results, bias):
    blocks = []
    for d in range(NCORES):
        numT = results[d]["numT"].astype(np.float64)   # [C_OUT, JB]
        den = results[d]["den"].astype(np.float64)     # [1, JB]
        blocks.append((numT / den).T)
    out = np.concatenate(blocks, axis=0) + bias.astype(np.float64)[None, :]
    return out.astype(np.float32)


def kernel(x, adj, W, att_src, att_dst, bias):
    nc = build_nc()
    in_maps = make_in_maps(x, adj, W, att_src, att_dst)
    res = run_bass_kernel_spmd(nc, in_maps, list(range(NCORES)))
    kernel._last_result = res
    return postprocess(res.results, bias)



# revision 30
# speedup vs baseline: 1.3661x; 1.3661x over previous
"""DenseGATConv Trainium2 kernel v3 (8 NeuronCores, SPMD, column-sharded).

Math (per core, owning JB=1024 destination columns j):
    u_i = exp(0.2 a_src_i), e_i = exp(a_src_i), q_j = exp(0.8 a_dst_j)
    M[i,j] = adj[i,j] * max(e_i q_j, u_i)
    out[j,:] = (M^T h)[j,:] / colsum(M)[j] + bias.

v3 structural change vs v2: the masked matmul uses m as lhsT per j-tile
(128 columns) against rhs = (h | 1) with 130 columns, accumulating
PSUM[j, 130] over all 64 i-tiles. Column 129 of the accumulator is then
colsum(M) -- the denominator comes FREE with the numerator matmul, so all
den-specific work (DVE tree-adds + PE ones-matmuls, ~29us of engine time)
is gone, and the output is produced directly in [j, c] orientation.
"""

import os
import numpy as np
import ml_dtypes
from contextlib import ExitStack

import concourse.bass as bass
import concourse.bacc as bacc
import concourse.tile as tile
from concourse import mybir
from concourse.bass_utils import run_bass_kernel_spmd

F32 = mybir.dt.float32
F16 = mybir.dt.float16
F8 = mybir.dt.float8e4
ALU = mybir.AluOpType
ACTF = mybir.ActivationFunctionType

N, C_IN, C_OUT = 8192, 256, 128
NCORES = 8
JB = N // NCORES          # 1024 destination columns per core
NT = N // 128             # 64 i-tiles
QUAD = 4                  # i-tiles per adj DMA chunk / per mask-mult op
NQ = NT // QUAD           # 16 quads
GRP = 8                   # a_src exp-group size (i-tiles)
XB = 16                   # i-tiles per xT chunk
NXC = NT // XB            # 4 xT chunks
NJT = JB // 128           # 8 j-tiles per core
_ABL = os.environ.get("KABLATE", "")
POOL_Q = (1, 4, 7, 10, 13)  # quads whose mask-mult runs on Pool, adj in fp8
if _ABL == "nopool":
    POOL_Q = ()
POOL_F16 = _ABL == "poolf16"     # pool quads use f16 adj (tests gpsimd TT)
POOL_DVE = _ABL == "pooldve"     # pool quads masked on DVE (tests fp8 TT)
NO_DEFER = _ABL == "nodefer"     # no deferred matmul emission
KDEBUG = bool(os.environ.get("KDEBUG"))

_nc_cache = {}


def _make_pools(tc, ctx):
    return dict(
        const=ctx.enter_context(tc.tile_pool(name="const", bufs=2)),
        xt_pool=ctx.enter_context(tc.tile_pool(name="xt", bufs=3)),
        h_pool=ctx.enter_context(tc.tile_pool(name="h", bufs=1)),
        persist=ctx.enter_context(tc.tile_pool(name="persist", bufs=1)),
        adj_pool=ctx.enter_context(tc.tile_pool(name="adj", bufs=6)),
        adj8_pool=ctx.enter_context(tc.tile_pool(name="adj8", bufs=3)),
        t2_pool=ctx.enter_context(tc.tile_pool(name="t2", bufs=2)),
        t2p_pool=ctx.enter_context(tc.tile_pool(name="t2p", bufs=3)),
        m_pool=ctx.enter_context(tc.tile_pool(name="m", bufs=3)),
        mp_pool=ctx.enter_context(tc.tile_pool(name="mp", bufs=3)),
        ps_h=ctx.enter_context(tc.tile_pool(name="psh", bufs=2, space="PSUM")),
        ps_acc=ctx.enter_context(tc.tile_pool(name="psacc", bufs=1,
                                              space="PSUM")),
        ps_pre=ctx.enter_context(tc.tile_pool(name="pspre", bufs=1,
                                              space="PSUM")),
    )


def _emit_body(tc, nc, pools, tensors, rep):
    (xT_in, adj_in, adj8_in, W_in, out_out) = tensors
    cache = pools.setdefault("_cache", {})

    adj_r = adj_in.rearrange("(c a p) j -> c p a j", a=QUAD, p=128)
    adj8_r = adj8_in.rearrange("(c a p) j -> c p a j", a=QUAD, p=128)
    q2part = {}
    n16 = n8 = 0
    for q in range(NQ):
        if q in POOL_Q:
            q2part[q] = ("f8", n8); n8 += 1
        else:
            q2part[q] = ("f16", n16); n16 += 1

    const = pools["const"]
    xt_pool = pools["xt_pool"]
    h_pool = pools["h_pool"]
    persist = pools["persist"]
    adj_pool = pools["adj_pool"]
    adj8_pool = pools["adj8_pool"]
    t2_pool = pools["t2_pool"]
    t2p_pool = pools["t2p_pool"]
    mp_pool = pools["mp_pool"]
    m_pool = pools["m_pool"]
    ps_h = pools["ps_h"]
    ps_acc = pools["ps_acc"]
    ps_pre = pools["ps_pre"]

    # ---- front-loaded DMAs (SP queue, program order == stream order) ----
    W_sb = const.tile([128, 260], F16, tag="W_sb", name=f"W_sb_{rep}")
    nc.sync.dma_start(W_sb[:], W_in[:])
    wdst8 = W_sb[:, 258:260]

    xc = [xt_pool.tile([128, 2 * XB * 128], F16, tag="xtc", name=f"xc{cx}_{rep}")
          for cx in range(NXC)]

    xT_v = xT_in[:].rearrange("(two p) n -> p two n", two=2)

    def emit_xc_dma(cx):
        nc.sync.dma_start(
            xc[cx][:].rearrange("p (two n) -> p two n", two=2),
            xT_v[:, :, cx * XB * 128:(cx + 1) * XB * 128])

    # first half of xc0 (i-tiles 0..7, both k halves) lands first: those are
    # the core's OWN rows (host rotation), so a_dst comes from a small
    # direct matmul on xc0a with no extra x DMA.
    xc0v = xc[0][:].rearrange("p (two n) -> p two n", two=2)
    nc.sync.dma_start(xc0v[:, :, 0:XB * 64],
                      xT_v[:, :, 0:XB * 64])

    adj_tiles = []

    def emit_adj_dma(q, split=False):
        part, ci = q2part[q]
        dt = F16 if (part == "f16" or POOL_F16) else F8
        tag = "adj" if part == "f16" else "adj8"
        src_r = adj_r if part == "f16" else adj8_r
        pool_ = adj_pool if part == "f16" else adj8_pool
        adj_q = pool_.tile([128, QUAD * JB], dt, tag=tag,
                           name=f"adj{q}_{rep}")
        if split:
            half = src_r[ci][:, 0:QUAD // 2, :]
            nc.sync.dma_start(adj_q[:, 0:QUAD * JB // 2], half)
            nc.sync.dma_start(adj_q[:, QUAD * JB // 2:],
                              src_r[ci][:, QUAD // 2:QUAD, :])
        else:
            nc.sync.dma_start(adj_q[:], src_r[ci])
        adj_tiles.append(adj_q)

    # interleave adj and xc chunks; first quad split for an early start
    emit_adj_dma(0, split=True)
    nc.sync.dma_start(xc0v[:, :, XB * 64:XB * 128],
                      xT_v[:, :, XB * 64:XB * 128])
    emit_adj_dma(1)
    emit_adj_dma(2)
    emit_xc_dma(1)
    emit_adj_dma(3)
    emit_adj_dma(4)
    emit_adj_dma(5)
    emit_xc_dma(2)
    # adj quads 6..15 emitted in the main loop (pool bufs gate prefetch);
    # xc3 emitted after adj7 so a brief head-of-line stall cannot starve it


    # ---- device-side constants ----
    if "ones_row" not in cache:
        cache["ones_row"] = persist.tile([1, 128], F16, tag="ones_row",
                                         name="ones_row")
        nc.vector.memset(cache["ones_row"][:], 1.0)
    ones_row = cache["ones_row"]

    # ---- a_dst for the owned 1024 columns: small matmuls on xc0a (the
    # rotated-first own rows), then broadcast via ones matmul + exp(0.8x).
    q_rep = const.tile([128, JB], F16, tag="q_rep", name=f"q_rep_{rep}")
    adst_row = const.tile([1, JB], F16, tag="adst_row", name=f"adst_row_{rep}")

    def emit_adst_mms():
        for hf in range(2):
            ap = ps_pre.tile([1, 512], F32, tag="adst", name=f"adstp{hf}_{rep}")
            for k in range(2):
                nc.tensor.matmul(
                    ap[:], lhsT=wdst8[:, k:k + 1],
                    rhs=xc[0][:, k * XB * 128 + hf * 512:
                              k * XB * 128 + (hf + 1) * 512],
                    start=(k == 0), stop=(k == 1))
            nc.scalar.copy(adst_row[0:1, hf * 512:(hf + 1) * 512], ap[:])

    def emit_qrep_bcast():
        for hf in range(2):
            qp = ps_pre.tile([128, 512], F32, tag="qrep", name=f"qp{hf}_{rep}")
            nc.tensor.matmul(qp[:], lhsT=ones_row[:],
                             rhs=adst_row[0:1, hf * 512:(hf + 1) * 512],
                             start=True, stop=True)
            nc.scalar.activation(q_rep[:, hf * 512:(hf + 1) * 512], qp[:],
                                 ACTF.Exp, scale=0.8)

    # ---- h tiles + a_src (PE matmul w/ fused wsrc col) ----
    # h_t layout: [128, 130] = (h[0:128] | a_src | 1.0). The trailing ones
    # column makes the num matmul also produce colsum(M) in PSUM col 129.
    # Emission is interleaved with the quad loop (h stays ~3 quads ahead)
    # so every engine's in-order SEQ matches the dataflow order.
    h_tiles = [None] * NT
    asrc_g = [const.tile([128, GRP], F32, tag=f"asrc{g}", name=f"asrc{g}_{rep}")
              for g in range(NT // GRP)]
    ea_g = [const.tile([128, GRP], F32, tag=f"ea{g}", name=f"ea{g}_{rep}")
            for g in range(NT // GRP)]   # exp(a_src)
    u_g = [const.tile([128, GRP], F32, tag=f"u{g}", name=f"u{g}_{rep}")
           for g in range(NT // GRP)]    # exp(0.2 a_src)

    def emit_h(t):
        cx, ti = divmod(t, XB)
        g, gi = divmod(t, GRP)
        hp = ps_h.tile([128, 129], F32, tag="hps", name=f"hps{t}_{rep}")
        for k in range(2):
            nc.tensor.matmul(
                hp[:],
                lhsT=xc[cx][:, k * XB * 128 + ti * 128:
                            k * XB * 128 + (ti + 1) * 128],
                rhs=W_sb[:, k * 129:(k + 1) * 129],
                start=(k == 0), stop=(k == 1))
        hkey = f"h{t}"
        if hkey not in cache:
            cache[hkey] = h_pool.tile([128, 130], F16, tag=hkey,
                                      name=f"h{t}")
            nc.gpsimd.memset(cache[hkey][:, 129:130], 1.0)
        h_t = cache[hkey]
        nc.scalar.copy(asrc_g[g][:, gi:gi + 1], hp[:, 128:129])
        nc.scalar.copy(h_t[:, 0:129], hp[:])
        h_tiles[t] = h_t
        if gi == GRP - 1:
            nc.scalar.activation(ea_g[g][:], asrc_g[g][:], ACTF.Exp,
                                 scale=1.0)
            nc.scalar.activation(u_g[g][:], asrc_g[g][:], ACTF.Exp,
                                 scale=0.2)
        if t == 3:
            emit_qrep_bcast()

    H_AHEAD = 4               # quads of h-tile lead over the mask loop
    emit_adst_mms()
    for t in range(QUAD * H_AHEAD):
        emit_h(t)

    # ---- main masked-matmul loop (quad granularity) ----
    # PSUM: 4 banks, each holding two j-tile accumulators [128, 130] at
    # column offsets 0 and 256.
    num_ps = [ps_acc.tile([128, 512], F32, tag=f"nps{b}", name=f"nps{b}_{rep}")
              for b in range(4)]

    def acc_view(jt):
        return num_ps[jt // 2][:, (jt % 2) * 256:(jt % 2) * 256 + 130]

    half = QUAD * JB // 2
    # Pool-quad matmuls are deferred in PE emission order: Pool's mask-mult
    # is ~8us, so its matmuls are emitted a few quads later to give Pool a
    # head start, keeping PE stall-free. MM_AFTER[q] lists deferred quads
    # whose matmuls are emitted right after quad q's own.
    if POOL_Q and not NO_DEFER:
        MM_AFTER = {3: [1], 6: [4], 9: [7], 12: [10], 14: [13]}
    else:
        MM_AFTER = {}
    LAST_MM = []              # stop group rides the final quad (15)
    m_of = {}
    t2_of = {}

    # start=True zeroes the WHOLE 2KB psum bank (zero-region semantics), so
    # only the even-jt view (bank offset 0) starts its bank; the odd-jt
    # view's first start=False matmul lands on pending-zero bytes and
    # writes through instead of accumulating.
    def emit_mm(q, is_stop):
        if is_stop:
            # jt-major: each accumulator receives its stop as early as
            # possible so epilogue copies overlap the remaining matmuls
            for jt in range(NJT):
                for a in range(QUAD):
                    t = q * QUAD + a
                    nc.tensor.matmul(
                        acc_view(jt),
                        lhsT=m_of[q][:, a * JB + jt * 128:
                                     a * JB + (jt + 1) * 128],
                        rhs=h_tiles[t][:],
                        start=(t == 0 and jt % 2 == 0), stop=(a == QUAD - 1),
                        skip_group_check=True)
            return
        for a in range(QUAD):
            t = q * QUAD + a
            for jt in range(NJT):
                nc.tensor.matmul(
                    acc_view(jt),
                    lhsT=m_of[q][:, a * JB + jt * 128:a * JB + (jt + 1) * 128],
                    rhs=h_tiles[t][:],
                    start=(t == 0 and jt % 2 == 0),
                    stop=(is_stop and a == QUAD - 1),
                    skip_group_check=True)

    for q in range(NQ):
        if q + 6 < NQ:
            emit_adj_dma(q + 6)
        if q == 1:
            emit_xc_dma(3)
        if q + H_AHEAD < NQ:
            for a in range(QUAD):
                emit_h((q + H_AHEAD) * QUAD + a)
        adj_q = adj_tiles[q]

        def emit_t2(qq):
            tp_ = t2p_pool if qq in POOL_Q else t2_pool
            t2_qq = tp_.tile([128, QUAD * JB], F16,
                             tag="t2p" if qq in POOL_Q else "t2",
                             name=f"t2_{qq}_{rep}")
            for a in range(QUAD):
                t = qq * QUAD + a
                g, gi = divmod(t, GRP)
                nc.vector.tensor_scalar(
                    t2_qq[:, a * JB:(a + 1) * JB], q_rep[:],
                    ea_g[g][:, gi:gi + 1], u_g[g][:, gi:gi + 1],
                    op0=ALU.mult, op1=ALU.max)
            return t2_qq

        # pool-quad t2 is emitted two quads early so Pool's mask never waits
        if q == 0 and 1 in POOL_Q:
            t2_of[1] = emit_t2(1)
        if q + 2 in POOL_Q:
            t2_of[q + 2] = emit_t2(q + 2)
        if q in POOL_Q:
            t2_q = t2_of[q]
        else:
            t2_q = emit_t2(q)
        mpool_ = mp_pool if q in POOL_Q else m_pool
        m_q = mpool_.tile([128, QUAD * JB], F16,
                          tag="mp" if q in POOL_Q else "m",
                          name=f"m{q}_{rep}")
        m_of[q] = m_q
        if q in POOL_Q and not POOL_DVE:
            nc.gpsimd.tensor_tensor(m_q[:], t2_q[:], adj_q[:], op=ALU.mult)
        elif q in POOL_Q:
            nc.vector.tensor_tensor(m_q[:], t2_q[:], adj_q[:], op=ALU.mult)
        elif q == 0 or q == NQ - 1:
            nc.vector.tensor_tensor(m_q[:, 0:half], t2_q[:, 0:half],
                                    adj_q[:, 0:half], op=ALU.mult)
            nc.vector.tensor_tensor(m_q[:, half:], t2_q[:, half:],
                                    adj_q[:, half:], op=ALU.mult)
        else:
            nc.vector.tensor_tensor(m_q[:], t2_q[:], adj_q[:], op=ALU.mult)
        if q not in POOL_Q or NO_DEFER or not MM_AFTER:
            emit_mm(q, is_stop=(q == NQ - 1))
        for dq in MM_AFTER.get(q, []):
            emit_mm(dq, is_stop=False)
    for i, dq in enumerate(LAST_MM):
        emit_mm(dq, is_stop=(i == len(LAST_MM) - 1))

    if KDEBUG and rep == 0:
        nc.sync.dma_start(dbg_q[:], q_rep[:])
        nc.sync.dma_start(dbg_h[:], h_tiles[0][:])
        nc.sync.dma_start(dbg_m[:], m_of[0][:])
        nc.sync.dma_start(dbg_ea[:, 0:GRP], ea_g[0][:])
        nc.sync.dma_start(dbg_ea[:, GRP:2 * GRP], u_g[0][:])
        nc.sync.dma_start(dbg_adst[:], adst_row[:])

    # ---- epilogue: PSUM[j,130] -> (out | asrc-garbage | den-f16) ----
    if "out_sb" not in cache:
        cache["out_sb"] = persist.tile([128, NJT * 130], F16, tag="out_sb",
                                       name="out_sb")
    out_sb = cache["out_sb"]
    for jt in range(NJT):
        src = acc_view(jt)
        if jt % 2 == 0:
            nc.scalar.copy(out_sb[:, jt * 130:(jt + 1) * 130], src)
        else:
            nc.vector.tensor_copy(out_sb[:, jt * 130:(jt + 1) * 130], src)
    nc.sync.dma_start(out_out[:], out_sb[:])


def build_nc(reps=1):
    key = ("nc", reps)
    if key in _nc_cache:
        return _nc_cache[key]
    nc = bacc.Bacc("TRN2", target_bir_lowering=False, debug=False,
                   num_devices=NCORES)

    xT_in = nc.dram_tensor("xT", [C_IN, N], F16, kind="ExternalInput")
    adj_in = nc.dram_tensor("adjc", [(NQ - len(POOL_Q)) * 512, JB], F16,
                            kind="ExternalInput")
    adj8_in = nc.dram_tensor("adjc8", [max(1, len(POOL_Q)) * 512, JB],
                             F16 if POOL_F16 else F8,
                             kind="ExternalInput")
    W_in = nc.dram_tensor("Wt", [128, 260], F16, kind="ExternalInput")

    out_out = nc.dram_tensor("outj", [128, NJT * 130], F16,
                             kind="ExternalOutput")
    import os as _os
    if _os.environ.get("KDEBUG"):
        global dbg_q, dbg_h, dbg_m, dbg_ea, dbg_adst
        dbg_q = nc.dram_tensor("dbg_q", [128, JB], F16, kind="ExternalOutput")
        dbg_h = nc.dram_tensor("dbg_h", [128, 130], F16,
                               kind="ExternalOutput")
        dbg_m = nc.dram_tensor("dbg_m", [128, QUAD * JB], F16,
                               kind="ExternalOutput")
        dbg_ea = nc.dram_tensor("dbg_ea", [128, 2 * GRP], F32,
                                kind="ExternalOutput")
        dbg_adst = nc.dram_tensor("dbg_adst", [1, JB], F16,
                                  kind="ExternalOutput")

    tensors = (xT_in, adj_in, adj8_in, W_in, out_out)

    UNROLL = 4
    with tile.TileContext(nc) as tc:
        with ExitStack() as pctx:
            pools = _make_pools(tc, pctx)
            if reps >= 2 * UNROLL:
                n_loop, n_rem = divmod(reps, UNROLL)
                with tc.For_i(0, n_loop, 1, hint_engines=(
                        mybir.EngineType.PE, mybir.EngineType.DVE,
                        mybir.EngineType.Activation, mybir.EngineType.SP,
                        mybir.EngineType.Pool)):
                    for r in range(UNROLL):
                        _emit_body(tc, nc, pools, tensors, r)
                for r in range(n_rem):
                    _emit_body(tc, nc, pools, tensors, UNROLL + r)
            else:
                for r in range(reps):
                    _emit_body(tc, nc, pools, tensors, r)

    nc.compile()
    _nc_cache[key] = nc
    return nc


def make_in_maps(x, adj, W, att_src, att_dst):
    xT = np.ascontiguousarray(x.T.astype(np.float32, copy=False)).astype(
        np.float16)
    wsrc = (W.astype(np.float64) @ att_src.astype(np.float64))  # [256]
    wdst = (W.astype(np.float64) @ att_dst.astype(np.float64))  # [256]
    Wt = np.ascontiguousarray(np.concatenate(
        [W[0:128, :], wsrc[0:128, None], W[128:256, :], wsrc[128:256, None],
         wdst[0:128, None], wdst[128:256, None]],
        axis=1)).astype(np.float16)                             # [128, 260]
    f8 = ml_dtypes.float8_e4m3
    q16 = [q for q in range(NQ) if q not in POOL_Q]
    in_maps = []
    for d in range(NCORES):
        adj_d = np.ascontiguousarray(
            adj[:, d * JB:(d + 1) * JB].astype(np.float32, copy=False))
        idx = np.arange(JB)
        adj_d[d * JB + idx, idx] = 1.0          # self loops
        # rotate rows so the core's own j-block rows come first (the h
        # pipeline then yields a_dst for the owned columns from tiles 0..7)
        adj_d = np.concatenate([adj_d[d * JB:], adj_d[:d * JB]], axis=0)
        xT_d = np.ascontiguousarray(np.concatenate(
            [xT[:, d * JB:], xT[:, :d * JB]], axis=1))
        a16 = np.concatenate(
            [adj_d[q * 512:(q + 1) * 512] for q in q16], axis=0).astype(
            np.float16)                         # 0/1: exact
        a8dt = np.float16 if POOL_F16 else f8
        a8 = np.concatenate(
            [adj_d[q * 512:(q + 1) * 512] for q in POOL_Q] or
            [np.zeros((512, JB), np.float32)], axis=0).astype(a8dt)
        in_maps.append({
            "xT": xT_d, "adjc": np.ascontiguousarray(a16),
            "adjc8": np.ascontiguousarray(a8), "Wt": Wt,
        })
    return in_maps


def postprocess(results, bias):
    blocks = []
    for d in range(NCORES):
        oj = results[d]["outj"].astype(np.float64)   # [128, NJT*130]
        oj = oj.reshape(128, NJT, 130)
        num = np.transpose(oj[:, :, 0:C_OUT], (1, 0, 2))   # [NJT, 128, C]
        d_ = np.transpose(oj[:, :, 129:130], (1, 0, 2))    # [NJT, 128, 1]
        blocks.append((num / d_).reshape(JB, C_OUT))
    out = np.concatenate(blocks, axis=0) + bias.astype(np.float64)[None, :]
    return out.astype(np.float32)


def kernel(x, adj, W, att_src, att_dst, bias):
    nc = build_nc()
    in_maps = make_in_maps(x, adj, W, att_src, att_dst)
    res = run_bass_kernel_spmd(nc, in_maps, list(range(NCORES)))
    kernel._last_result = res
    return postprocess(res.results, bias)
